# revision 75
# baseline (speedup 1.0000x reference)
"""Trainium2 Bass kernel for OctahedralCavityProcessor.

Sharding: data-parallel over batch (B=8 -> 8 cores, zero collectives).
Each core processes one batch element b:
  phase A: cavity pooling  featT[c,k] = sum_p xT[p,c] * mask[p,k]
           (host-pre-transposed fp8 xT chunks accumulated straight into
           one PSUM bank; 1/count folded into W1 host-side)
  phase B: per-cavity MLP + 14-token multi-head attention (tiny, on-chip,
           batched matmuls, f32)
  phase C: out[c,p] = x[c,p] + ak[.,c] @ onehot[.,p]  (f16 matmul + add)

Geometry-only quantities (mask, onehot, counts) and all weight reshapes
are precomputed host-side in numpy; they do not depend on x.
"""

import numpy as np
import ml_dtypes

import concourse.bass as bass
import concourse.tile as tile
from concourse import mybir
from concourse.bass_utils import run_bass_kernel_spmd
from concourse.vector_clock import ScopedClock, VectorClock
from contextlib import ExitStack

F32 = mybir.dt.float32
F16 = mybir.dt.float16
F8 = mybir.dt.float8e4

NP_F8 = ml_dtypes.float8_e4m3

B, C, P, K, H = 8, 128, 100000, 14, 8
C2 = 2 * C
Dh = C // H
RADIUS = np.float32(0.5)

CHA = 128                     # phase A partition count
NA = (P + CHA - 1) // CHA     # 782 128-pt chunks
NAP = 784                     # padded 128-pt chunk count (mask zero on pad)
PT = NAP * CHA                # 100352 padded points
NAP2 = NAP // 2               # 392 256-pt DoubleRow chunks
KP = 16                       # K padded to 16 (DoubleRow 16B alignment)
GA = 49                       # 256-pt chunks per phase-A DMA group
NGA = NAP2 // GA              # 8 groups
CHC = 512                     # phase C psum chunk
CB = 4 * CHC                  # 2048-point phase C block
NB = PT // CB                 # 49 blocks

# wB column layout (partition dim C, f32)
W1_O = 0                      # [K*C2] w1t (inv-scaled)
W2_O = W1_O + K * C2          # [K*2*C] w2t halves
B1_O = W2_O + K * 2 * C       # [2K]
B2_O = B1_O + 2 * K           # [K]
WQ_O = B2_O + K               # [C]
WK_O = WQ_O + C               # [C]
WV_O = WK_O + C               # [C]
WB_COLS = WV_O + C


def _legalize_bir_waits(bir_json: bytes) -> bytes:
    """walrus here accepts at most ONE sync-wait command per instruction.
    Tile's scheduler may attach several.  Hoist the extras onto NoOp
    instructions inserted immediately before, on the same engine (the
    engine executes serially, so waiting one-at-a-time is equivalent)."""
    import json as _json

    d = _json.loads(bir_json)
    changed = False
    for fn in d.get("functions", []):
        for blk in fn.get("blocks", []):
            insts = blk.get("instructions", [])
            out = []
            for ins in insts:
                waits = (ins.get("sync_info") or {}).get("on_wait", [])
                if len(waits) > 1:
                    changed = True
                    for i, w in enumerate(waits[:-1]):
                        out.append({
                            "debug": ins.get("debug", 0),
                            "engine": ins["engine"],
                            "ins": [],
                            "name": f"{ins['name']}-wsplit{i}",
                            "opcode": "NoOp",
                            "outs": [],
                            "sync_info": {"on_update": [], "on_wait": [w]},
                            "text_hint": "wait_split",
                        })
                    ins["sync_info"]["on_wait"] = [waits[-1]]
                out.append(ins)
            blk["instructions"] = out
    if not changed:
        return bir_json
    return _json.dumps(d).encode()


def _install_wait_legalizer():
    import concourse.bass2jax as _b2j

    orig = _b2j.compile_bir_kernel
    if getattr(orig, "_wait_legalized", False):
        return

    def patched(bir_json, tmpdir, neff_name="file.neff"):
        return orig(_legalize_bir_waits(bir_json), tmpdir, neff_name=neff_name)

    patched._wait_legalized = True
    _b2j.compile_bir_kernel = patched


_install_wait_legalizer()


class SplitDrainTileContext(tile.TileContext):
    """The walrus build here only accepts ONE sync-wait command per
    instruction; stock TileContext puts every live sem wait on the tail
    Drain.  Split them across nop instructions instead."""

    def _drain_and_barrier(self, tick_clock, wait_clock):
        gc = tick_clock.global_clock
        n = len(gc)
        for i in range(n):
            if gc[i] <= 0:
                continue
            vec = [gc[j] if j == i else 0 for j in range(n)]
            nop = self.nc.sync.nop(nofuse=True, hint="tail_drain_split")
            wait_clock.add_sem_waits(nop.ins, ScopedClock({None: VectorClock(vec)}))
        self.nc.sync.drain()
        self.nc.all_engine_barrier()
        assert self.sems is not None
        popped = self.nc._tile_sem_poison_stack.pop()
        assert popped is self._sem_poison
        self.nc.clear_and_free_semaphores(list(self.sems.allocated().values()))
        self.nc.all_engine_barrier()


def build_program(reps=1):
    nc = bass.Bass()

    xT_d = nc.dram_tensor("xT", [CHA, NAP2 * 2 * C], F8, kind="ExternalInput")
    mask_d = nc.dram_tensor("maskA", [CHA, NAP2 * 2 * KP], F8,
                            kind="ExternalInput")
    xc_d = nc.dram_tensor("xc", [C, PT], F16, kind="ExternalInput")
    onehot_d = nc.dram_tensor("onehot", [K, PT], F8, kind="ExternalInput")
    identh_d = nc.dram_tensor("identh", [C, C], F16, kind="ExternalInput")
    wB_d = nc.dram_tensor("wB", [C, WB_COLS], F32, kind="ExternalInput")
    wS_d = nc.dram_tensor("wS", [Dh, H * C + 2 * H], F32, kind="ExternalInput")
    vb_d = nc.dram_tensor("vb", [K, C], F32, kind="ExternalInput")
    ob_d = nc.dram_tensor("ob", [C, 1], F32, kind="ExternalInput")
    ident_d = nc.dram_tensor("ident", [C, C], F32, kind="ExternalInput")
    out_d = nc.dram_tensor("out", [C, P], F16, kind="ExternalOutput")

    Id = mybir.ActivationFunctionType.Identity

    with SplitDrainTileContext(nc) as tc:
      for _rep in range(reps):
        with ExitStack() as octx:
            cpool = octx.enter_context(tc.tile_pool(name="consts", bufs=1))

            # ---- mask first (gates phase A), then weights (needed in B) ----
            m_pool = octx.enter_context(tc.tile_pool(name="mA", bufs=1))
            MHALF = (NAP2 // 2) * 2 * KP
            m_all0 = m_pool.tile([CHA, MHALF], F8, tag="m0")
            nc.scalar.dma_start(m_all0[:], mask_d[:, :MHALF])
            m_all1 = m_pool.tile([CHA, MHALF], F8, tag="m1")
            nc.scalar.dma_start(m_all1[:], mask_d[:, MHALF:])
            abctx = octx.enter_context(ExitStack())
            xg_pool = abctx.enter_context(tc.tile_pool(name="xg", bufs=3))
            C2W = 2 * C   # 256 cols per DoubleRow chunk
            K2W = 2 * KP
            xg_tiles = {}
            wB_s = cpool.tile([C, WB_COLS], F32, tag="wB")
            nc.scalar.dma_start(wB_s[:], wB_d[:])
            wS_s = cpool.tile([Dh, H * C + 2 * H], F32, tag="wS")
            nc.scalar.dma_start(wS_s[:], wS_d[:])
            vb_s = cpool.tile([K, C], F32, tag="vb")
            nc.scalar.dma_start(vb_s[:], vb_d[:])
            ob_s = cpool.tile([C, 1], F32, tag="ob")
            nc.scalar.dma_start(ob_s[:], ob_d[:])
            ident_s = cpool.tile([C, C], F32, tag="ident")
            nc.scalar.dma_start(ident_s[:], ident_d[:])
            identh_s = cpool.tile([C, C], F16, tag="identh")
            nc.scalar.dma_start(identh_s[:], identh_d[:])
            # warm the Act tanh table while phase A runs
            tanh_warm = cpool.tile([C, 1], F32, tag="tanhw")
            nc.scalar.activation(tanh_warm[:], ob_s[:],
                                 mybir.ActivationFunctionType.Tanh)

            # ---------------- phase A: cavity pooling ----------------
            # featT[c,k] accumulated over 784 point-chunks in one PSUM bank.
            apool = abctx.enter_context(tc.tile_pool(name="a_ps", bufs=1,
                                                     space="PSUM"))
            featT_ps = apool.tile([C, KP], F32, tag="featT")
            with ExitStack() as actx:
                for g in range(NGA):
                    if g in xg_tiles:
                        xg_t = xg_tiles.pop(g)
                    else:
                        xg_t = xg_pool.tile([CHA, GA * C2W], F8, tag="xg")
                        eng = nc.sync if g % 2 == 0 else nc.gpsimd
                        eng.dma_start(
                            xg_t[:], xT_d[:, g * GA * C2W:(g + 1) * GA * C2W])
                    for j in range(GA):
                        ci = g * GA + j
                        mh, mi = divmod(ci, NAP2 // 2)
                        m_t = m_all0 if mh == 0 else m_all1
                        nc.tensor.matmul(
                            featT_ps[:],
                            lhsT=xg_t[:, j * C2W:(j + 1) * C2W]
                            .rearrange("p (i c) -> p i c", i=2),
                            rhs=m_t[:, mi * K2W:(mi + 1) * K2W]
                            .rearrange("p (i k) -> p i k", i=2),
                            start=(ci == 0),
                            stop=(ci == NAP2 - 1),
                            perf_mode=mybir.MatmulPerfMode.DoubleRow,
                        )

            # ---------------- phase B: MLP + attention (f32) ----------------
            with ExitStack() as bctx:
                sp = bctx.enter_context(
                    tc.tile_pool(name="sp_ps", bufs=2, space="PSUM"))

                featT = cpool.tile([C, K], F32, tag="featT_s")
                nc.vector.tensor_copy(featT[:], featT_ps[:, :K])

                # MLP: ph_all[:, 2k+h] = W1[k,h-half]^T @ featT[:,k]
                ph_all = sp.tile([C, 2 * K], F32, tag="sps")
                for k in range(K):
                    for hf in range(2):
                        nc.tensor.matmul(
                            ph_all[:, 2 * k + hf:2 * k + hf + 1],
                            lhsT=wB_s[:, W1_O + k * C2 + hf * C:
                                      W1_O + k * C2 + (hf + 1) * C],
                            rhs=featT[:, k:k + 1],
                            start=(k == 0 and hf == 0),
                            stop=(k == K - 1 and hf == 1),
                        )
                h_all = cpool.tile([C, 2 * K], F32, tag="h_all")
                nc.vector.tensor_add(h_all[:], ph_all[:],
                                     wB_s[:, B1_O:B1_O + 2 * K])
                nc.vector.tensor_scalar_max(h_all[:], h_all[:], 0.0)

                pp_all = sp.tile([C, K], F32, tag="sps")
                for k in range(K):
                    for hf in range(2):
                        nc.tensor.matmul(
                            pp_all[:, k:k + 1],
                            lhsT=wB_s[:, W2_O + (2 * k + hf) * C:
                                      W2_O + (2 * k + hf + 1) * C],
                            rhs=h_all[:, 2 * k + hf:2 * k + hf + 1],
                            start=(k == 0 and hf == 0),
                            stop=(k == K - 1 and hf == 1),
                        )
                pb_all = cpool.tile([C, K], F32, tag="pb_all")
                nc.vector.tensor_add(pb_all[:], pp_all[:],
                                     wB_s[:, B2_O:B2_O + K])
                procT = cpool.tile([C, K], F32, tag="procT")
                nc.scalar.activation(procT[:], pb_all[:],
                                     mybir.ActivationFunctionType.Tanh)

                # ---- attention over K=14 cavities ----
                # q/k in head-blocked layout [Dh, H*K]
                pq = sp.tile([Dh, H * K], F32, tag="sps")
                for h in range(H):
                    nc.tensor.matmul(pq[:, h * K:(h + 1) * K],
                                     lhsT=wB_s[:, WQ_O + h * Dh:
                                               WQ_O + (h + 1) * Dh],
                                     rhs=procT[:],
                                     start=(h == 0), stop=(h == H - 1))
                qh_s = cpool.tile([Dh, H * K], F32, tag="qT")
                for h in range(H):
                    nc.vector.tensor_scalar_add(
                        qh_s[:, h * K:(h + 1) * K],
                        pq[:, h * K:(h + 1) * K],
                        wS_s[:, H * C + h:H * C + h + 1])

                pk = sp.tile([Dh, H * K], F32, tag="sps")
                for h in range(H):
                    nc.tensor.matmul(pk[:, h * K:(h + 1) * K],
                                     lhsT=wB_s[:, WK_O + h * Dh:
                                               WK_O + (h + 1) * Dh],
                                     rhs=procT[:],
                                     start=(h == 0), stop=(h == H - 1))
                kh_s = cpool.tile([Dh, H * K], F32, tag="kT")
                for h in range(H):
                    nc.vector.tensor_scalar_add(
                        kh_s[:, h * K:(h + 1) * K],
                        pk[:, h * K:(h + 1) * K],
                        wS_s[:, H * C + H + h:H * C + H + h + 1])

                pv = sp.tile([K, C], F32, tag="sps")
                nc.tensor.matmul(pv[:], lhsT=procT[:],
                                 rhs=wB_s[:, WV_O:WV_O + C])
                v_s = cpool.tile([K, C], F32, tag="v")
                nc.vector.tensor_add(v_s[:], pv[:], vb_s[:])

                psc = sp.tile([K, H * K], F32, tag="sps")
                for h in range(H):
                    nc.tensor.matmul(
                        psc[:, h * K:(h + 1) * K],
                        lhsT=qh_s[:, h * K:(h + 1) * K],
                        rhs=kh_s[:, h * K:(h + 1) * K],
                        start=(h == 0),
                        stop=(h == H - 1),
                    )
                negmax = cpool.tile([K, H], F32, tag="negmax")
                nc.vector.tensor_reduce(
                    out=negmax[:],
                    in_=psc[:].rearrange("p (h j) -> p h j", j=K),
                    op=mybir.AluOpType.max,
                    axis=mybir.AxisListType.X,
                    negate=True,
                )
                esc = cpool.tile([K, H * K], F32, tag="esc")
                for h in range(H):
                    nc.scalar.activation(
                        esc[:, h * K:(h + 1) * K], psc[:, h * K:(h + 1) * K],
                        mybir.ActivationFunctionType.Exp,
                        bias=negmax[:, h:h + 1],
                    )
                ssum = cpool.tile([K, H], F32, tag="ssum")
                nc.vector.tensor_reduce(
                    out=ssum[:],
                    in_=esc[:].rearrange("p (h j) -> p h j", j=K),
                    op=mybir.AluOpType.add,
                    axis=mybir.AxisListType.X,
                )
                rinv = cpool.tile([K, H], F32, tag="rinv")
                nc.vector.reciprocal(rinv[:], ssum[:])
                for h in range(H):
                    nc.vector.tensor_scalar_mul(
                        esc[:, h * K:(h + 1) * K], esc[:, h * K:(h + 1) * K],
                        rinv[:, h:h + 1],
                    )

                pat = sp.tile([K, H * K], F32, tag="sps")
                for h in range(H):
                    nc.tensor.matmul(
                        pat[:, h * K:(h + 1) * K],
                        lhsT=esc[:, h * K:(h + 1) * K],
                        rhs=ident_s[:K, :K],
                        is_transpose=True,
                        start=(h == 0),
                        stop=(h == H - 1),
                    )
                at_s = cpool.tile([K, H * K], F32, tag="at")
                nc.vector.tensor_copy(at_s[:], pat[:])

                # o in head-blocked layout [Dh, H*K]
                po = sp.tile([Dh, H * K], F32, tag="sps")
                for h in range(H):
                    nc.tensor.matmul(
                        po[:, h * K:(h + 1) * K],
                        lhsT=v_s[:, h * Dh:(h + 1) * Dh],
                        rhs=at_s[:, h * K:(h + 1) * K],
                        start=(h == 0),
                        stop=(h == H - 1),
                    )
                o_s = cpool.tile([Dh, H * K], F32, tag="o")
                nc.vector.tensor_copy(o_s[:], po[:])

                # attT[e,i] = sum_h Wo[:, h-block] @ o_head_h  (accumulate)
                patt = sp.tile([C, K], F32, tag="sps")
                for h in range(H):
                    nc.tensor.matmul(patt[:],
                                     lhsT=wS_s[:, h * C:(h + 1) * C],
                                     rhs=o_s[:, h * K:(h + 1) * K],
                                     start=(h == 0), stop=(h == H - 1))
                attT_s = cpool.tile([C, K], F32, tag="attT")
                nc.vector.tensor_scalar_add(attT_s[:], patt[:], ob_s[:])

                pak = sp.tile([K, C], F32, tag="sps")
                nc.tensor.transpose(pak[:], attT_s[:], ident_s[:])
                ak_s = cpool.tile([K, C], F16, tag="ak")
                nc.vector.tensor_copy(ak_s[:], pak[:])
            abctx.close()   # free phase A/B PSUM banks for phase C

            # ---------------- phase C: gather-add ----------------
            # 4096-pt xc/out DMA super-blocks (Pool / SP queues), 8192-pt
            # fp8 onehot batches (Act queue).  Per 512-pt chunk: gather
            # matmul; chunks 0,1 add x on DVE straight from PSUM, chunks
            # 2,3 accumulate x on PE (identity matmul) and copy out on Act.
            OB = 2 * CB              # 4096
            NSB = (NB + 1) // 2      # 25, last super-block has 1 block
            OHB = 4 * CB             # 8192
            with ExitStack() as cctx:
                xc_pool = cctx.enter_context(tc.tile_pool(name="xc", bufs=7))
                oh_pool = cctx.enter_context(tc.tile_pool(name="oh", bufs=4))
                oc_pool = cctx.enter_context(tc.tile_pool(name="oc", bufs=4))
                pc_pool = cctx.enter_context(
                    tc.tile_pool(name="pc", bufs=4, space="PSUM"))

                NOHB = (NB + 3) // 4          # 13 onehot batches
                LA = 5                        # xc load lookahead

                def issue_xc(i):
                    s0 = i * OB
                    sw = min(OB, PT - s0)
                    t = xc_pool.tile([C, OB], F16, tag="xc")
                    nc.gpsimd.dma_start(t[:, :sw], xc_d[:, s0:s0 + sw])
                    return t

                def issue_oh(g):
                    o0 = g * OHB
                    ow = min(OHB, PT - o0)
                    t = oh_pool.tile([K, OHB], F8, tag="oh")
                    nc.scalar.dma_start(t[:, :ow], onehot_d[:, o0:o0 + ow])
                    return t

                xc_tiles = {i: issue_xc(i) for i in range(min(LA, NSB))}
                oh_tiles = {g: issue_oh(g) for g in range(min(3, NOHB))}

                for sb in range(NSB):
                    if sb + LA < NSB:
                        xc_tiles[sb + LA] = issue_xc(sb + LA)
                    s0 = sb * OB
                    xc_t = xc_tiles.pop(sb)
                    oc_t = oc_pool.tile([C, OB], F16, tag="ocd")
                    for jb in (0, 1):
                        b0 = s0 + jb * CB
                        if b0 >= P:
                            break
                        gi = sb * 2 + jb
                        g = gi // 4
                        if g + 3 < NOHB and g + 3 not in oh_tiles and gi % 4 == 0:
                            oh_tiles[g + 3] = issue_oh(g + 3)
                        oh_t = oh_tiles[g]
                        ohoff = (gi % 4) * CB
                        xoff = jb * CB
                        # two-bank [C, 1024] psum tiles: one DVE add (chunks
                        # 0-1) and one Act copy (chunks 2-3) per 2048-pt block
                        for j2 in (0, 1):
                            j0 = j2 * 2 * CHC
                            pc_t = pc_pool.tile([C, 2 * CHC], F32, tag="pc")
                            for jj in (0, 1):
                                j = j0 + jj * CHC
                                nc.tensor.matmul(
                                    pc_t[:, jj * CHC:(jj + 1) * CHC],
                                    lhsT=ak_s[:],
                                    rhs=oh_t[:, ohoff + j:ohoff + j + CHC],
                                    start=True, stop=(j2 == 0))
                                if j2 == 1:
                                    nc.tensor.matmul(
                                        pc_t[:, jj * CHC:(jj + 1) * CHC],
                                        lhsT=identh_s[:],
                                        rhs=xc_t[:, xoff + j:xoff + j + CHC],
                                        start=False, stop=True)
                            if j2 == 0:
                                nc.vector.tensor_add(
                                    oc_t[:, xoff + j0:xoff + j0 + 2 * CHC],
                                    pc_t[:],
                                    xc_t[:, xoff + j0:xoff + j0 + 2 * CHC])
                            else:
                                nc.scalar.activation(
                                    oc_t[:, xoff + j0:xoff + j0 + 2 * CHC],
                                    pc_t[:], Id)
                        bw = min(CB, P - b0)
                        nc.sync.dma_start(out_d[:, b0:b0 + bw],
                                          oc_t[:, xoff:xoff + bw])

    return nc


def prep_host(points, cavities, w1, b1, w2, b2, in_w, in_b, out_w, out_b):
    """Geometry + weight preprocessing (pure numpy, no x dependence)."""
    points = np.asarray(points, np.float32)
    cavities = np.asarray(cavities, np.float32)
    d = np.sqrt(
        ((points[None, :, :] - cavities[:, None, :]) ** 2).sum(-1, dtype=np.float32)
    ).astype(np.float32)                                   # [K, P]
    mask = (d < RADIUS).astype(np.float32)                 # [K, P]
    counts = mask.sum(axis=1, dtype=np.float32)            # [K]
    inv = np.where(counts > 0, 1.0 / np.maximum(counts, 1.0), 0.0).astype(np.float32)

    # DoubleRow mask: maskA[p, ci*2KP + i*KP + k] = mask[k, ci*256 + 2p + i]
    maskA = np.zeros((PT, KP), np.float32)
    maskA[:P, :K] = mask.T
    maskA = (maskA.reshape(NAP2, CHA, 2, KP).transpose(1, 0, 2, 3)
             .reshape(CHA, NAP2 * 2 * KP)).astype(NP_F8)

    nearest = np.argmin(d, axis=0)                         # [P]
    onehot = np.zeros((K, PT), NP_F8)
    onehot[nearest, np.arange(P)] = 1.0

    w1 = np.asarray(w1, np.float32)
    w2 = np.asarray(w2, np.float32)
    scale = np.float32(1.0 / np.sqrt(Dh))

    wB = np.zeros((C, WB_COLS), np.float32)
    # w1t scaled by inv[k]: wB[c, W1_O + k*C2 + d] = w1[k, d, c] * inv[k]
    w1t = (w1.transpose(0, 2, 1) * inv[:, None, None]).astype(np.float32)  # [K,C,C2]
    wB[:, W1_O:W1_O + K * C2] = w1t.transpose(1, 0, 2).reshape(C, K * C2)
    # w2t halves: wB[d, W2_O + (2k+h)*C + c] = w2[k, c, h*C+d]
    w2t = w2.transpose(0, 2, 1).reshape(K, 2, C, C)        # [K,2,C(d),C(c)]
    wB[:, W2_O:W2_O + K * 2 * C] = w2t.transpose(2, 0, 1, 3).reshape(C, K * 2 * C)
    b1 = np.asarray(b1, np.float32)
    wB[:, B1_O:B1_O + 2 * K] = b1.reshape(K, 2, C).transpose(2, 0, 1).reshape(C, 2 * K)
    wB[:, B2_O:B2_O + K] = np.asarray(b2, np.float32).T
    in_w = np.asarray(in_w, np.float32)
    wB[:, WQ_O:WQ_O + C] = in_w[0:C].T * scale
    wB[:, WK_O:WK_O + C] = in_w[C:2 * C].T
    wB[:, WV_O:WV_O + C] = in_w[2 * C:3 * C].T

    in_b = np.asarray(in_b, np.float32)
    # wS: wo_heads[d, h*C+e] = out_w[e, h*Dh+d], then qb/kb head-blocked
    wS = np.concatenate([
        np.asarray(out_w, np.float32).reshape(C, H, Dh)
        .transpose(2, 1, 0).reshape(Dh, H * C),
        (in_b[0:C] * scale).reshape(H, Dh).T,              # [Dh, H]
        in_b[C:2 * C].reshape(H, Dh).T,
    ], axis=1)                                             # [Dh, H*C+2H]

    fp = {
        "maskA": np.ascontiguousarray(maskA),
        "onehot": np.ascontiguousarray(onehot),
        "wB": np.ascontiguousarray(wB),
        "wS": np.ascontiguousarray(wS),
        "vb": np.ascontiguousarray(np.tile(in_b[2 * C:3 * C], (K, 1))),
        "ob": np.ascontiguousarray(np.asarray(out_b, np.float32).reshape(C, 1)),
        "ident": np.eye(C, dtype=np.float32),
        "identh": np.eye(C, dtype=np.float16),
    }
    return fp


def prep_x(xb):
    """Per-batch x prep: fp8 DoubleRow-transposed copy (phase A) + f16 copy
    (phase C).  xT[p, ci*2C + i*C + c] = x[c, ci*256 + 2p + i]."""
    xb = np.asarray(xb, np.float32)
    xpad = np.zeros((C, PT), np.float32)
    xpad[:, :P] = xb
    xT = np.ascontiguousarray(
        xpad.reshape(C, NAP2, CHA, 2).transpose(2, 1, 3, 0)
    ).reshape(CHA, NAP2 * 2 * C)
    return {
        "xT": xT.astype(NP_F8),
        "xc": xpad.astype(np.float16),
    }


def make_in_maps(x, fp):
    x = np.asarray(x, np.float32)
    return [dict(fp, **prep_x(x[b])) for b in range(B)]


_PROGRAM = None


def kernel(x, points, cavities, w1, b1, w2, b2, in_w, in_b, out_w, out_b):
    global _PROGRAM
    fp = prep_host(points, cavities, w1, b1, w2, b2, in_w, in_b, out_w, out_b)
    if _PROGRAM is None:
        _PROGRAM = build_program()
    nc = _PROGRAM
    in_maps = make_in_maps(x, fp)
    res = run_bass_kernel_spmd(nc, in_maps, list(range(B)))
    out = np.stack([np.asarray(res.results[b]["out"], np.float32)
                    for b in range(B)], axis=0)
    return out


# revision 76
# speedup vs baseline: 5.0000x; 5.0000x over previous
"""Trainium2 Bass kernel for OctahedralCavityProcessor.

Sharding: data-parallel over batch (B=8 -> 8 cores, zero collectives).
Each core processes one batch element b in three pipelined phases:
  phase A: cavity pooling  featT[c,k] = sum_p xT[p,c] * mask[p,k].
           x is host-pre-transposed to fp8 in DoubleRow-packed layout
           (256 points per matmul), mask is 0/1 fp8 with 1/count folded
           into W1 host-side; 392 matmuls accumulate one PSUM bank.
           xT streams on the SP+Pool DMA queues, mask/weights on Act.
  phase B: per-cavity MLP + 14-token multi-head attention, on-chip f32;
           batched matmuls (one PSUM accumulation group per stage),
           activation-table thrash avoided by doing bias-adds on DVE
           and warming the tanh table during phase A.
  phase C: out[c,p] = x[c,p] + ak[.,c] @ onehot[.,p].  f16 x / fp8
           onehot stream in 2048-pt blocks (xc on Pool, out on SP,
           onehot batched x4 on Act, software-pipelined lookahead);
           per block one [C,1024] two-bank PSUM DVE add and one Act
           copy (x pre-accumulated on PE via an identity matmul).

Geometry-only quantities (mask, onehot, counts) and all weight reshapes
are precomputed host-side in numpy; they do not depend on x.

Measured via the reps-in-one-NEFF slope method (bench2.py, device-
resident inputs): ~0.27-0.30 ms per iteration; no-exec CoreSim cost
model: 130 us.  Baseline staged at session start: ~1.5 ms.
"""

import numpy as np
import ml_dtypes

import concourse.bass as bass
import concourse.tile as tile
from concourse import mybir
from concourse.bass_utils import run_bass_kernel_spmd
from concourse.vector_clock import ScopedClock, VectorClock
from contextlib import ExitStack

F32 = mybir.dt.float32
F16 = mybir.dt.float16
F8 = mybir.dt.float8e4

NP_F8 = ml_dtypes.float8_e4m3

B, C, P, K, H = 8, 128, 100000, 14, 8
C2 = 2 * C
Dh = C // H
RADIUS = np.float32(0.5)

CHA = 128                     # phase A partition count
NA = (P + CHA - 1) // CHA     # 782 128-pt chunks
NAP = 784                     # padded 128-pt chunk count (mask zero on pad)
PT = NAP * CHA                # 100352 padded points
NAP2 = NAP // 2               # 392 256-pt DoubleRow chunks
KP = 16                       # K padded to 16 (DoubleRow 16B alignment)
GA = 49                       # 256-pt chunks per phase-A DMA group
NGA = NAP2 // GA              # 8 groups
CHC = 512                     # phase C psum chunk
CB = 4 * CHC                  # 2048-point phase C block
NB = PT // CB                 # 49 blocks

# wB column layout (partition dim C, f32)
W1_O = 0                      # [K*C2] w1t (inv-scaled)
W2_O = W1_O + K * C2          # [K*2*C] w2t halves
B1_O = W2_O + K * 2 * C       # [2K]
B2_O = B1_O + 2 * K           # [K]
WQ_O = B2_O + K               # [C]
WK_O = WQ_O + C               # [C]
WV_O = WK_O + C               # [C]
WB_COLS = WV_O + C


def _legalize_bir_waits(bir_json: bytes) -> bytes:
    """walrus here accepts at most ONE sync-wait command per instruction.
    Tile's scheduler may attach several.  Hoist the extras onto NoOp
    instructions inserted immediately before, on the same engine (the
    engine executes serially, so waiting one-at-a-time is equivalent)."""
    import json as _json

    d = _json.loads(bir_json)
    changed = False
    for fn in d.get("functions", []):
        for blk in fn.get("blocks", []):
            insts = blk.get("instructions", [])
            out = []
            for ins in insts:
                waits = (ins.get("sync_info") or {}).get("on_wait", [])
                if len(waits) > 1:
                    changed = True
                    for i, w in enumerate(waits[:-1]):
                        out.append({
                            "debug": ins.get("debug", 0),
                            "engine": ins["engine"],
                            "ins": [],
                            "name": f"{ins['name']}-wsplit{i}",
                            "opcode": "NoOp",
                            "outs": [],
                            "sync_info": {"on_update": [], "on_wait": [w]},
                            "text_hint": "wait_split",
                        })
                    ins["sync_info"]["on_wait"] = [waits[-1]]
                out.append(ins)
            blk["instructions"] = out
    if not changed:
        return bir_json
    return _json.dumps(d).encode()


def _install_wait_legalizer():
    import concourse.bass2jax as _b2j

    orig = _b2j.compile_bir_kernel
    if getattr(orig, "_wait_legalized", False):
        return

    def patched(bir_json, tmpdir, neff_name="file.neff"):
        return orig(_legalize_bir_waits(bir_json), tmpdir, neff_name=neff_name)

    patched._wait_legalized = True
    _b2j.compile_bir_kernel = patched


_install_wait_legalizer()


class SplitDrainTileContext(tile.TileContext):
    """The walrus build here only accepts ONE sync-wait command per
    instruction; stock TileContext puts every live sem wait on the tail
    Drain.  Split them across nop instructions instead."""

    def _drain_and_barrier(self, tick_clock, wait_clock):
        gc = tick_clock.global_clock
        n = len(gc)
        for i in range(n):
            if gc[i] <= 0:
                continue
            vec = [gc[j] if j == i else 0 for j in range(n)]
            nop = self.nc.sync.nop(nofuse=True, hint="tail_drain_split")
            wait_clock.add_sem_waits(nop.ins, ScopedClock({None: VectorClock(vec)}))
        self.nc.sync.drain()
        self.nc.all_engine_barrier()
        assert self.sems is not None
        popped = self.nc._tile_sem_poison_stack.pop()
        assert popped is self._sem_poison
        self.nc.clear_and_free_semaphores(list(self.sems.allocated().values()))
        self.nc.all_engine_barrier()


def build_program(reps=1):
    nc = bass.Bass()

    xT_d = nc.dram_tensor("xT", [CHA, NAP2 * 2 * C], F8, kind="ExternalInput")
    mask_d = nc.dram_tensor("maskA", [CHA, NAP2 * 2 * KP], F8,
                            kind="ExternalInput")
    xc_d = nc.dram_tensor("xc", [C, PT], F16, kind="ExternalInput")
    onehot_d = nc.dram_tensor("onehot", [K, PT], F8, kind="ExternalInput")
    identh_d = nc.dram_tensor("identh", [C, C], F16, kind="ExternalInput")
    wB_d = nc.dram_tensor("wB", [C, WB_COLS], F32, kind="ExternalInput")
    wS_d = nc.dram_tensor("wS", [Dh, H * C + 2 * H], F32, kind="ExternalInput")
    vb_d = nc.dram_tensor("vb", [K, C], F32, kind="ExternalInput")
    ob_d = nc.dram_tensor("ob", [C, 1], F32, kind="ExternalInput")
    ident_d = nc.dram_tensor("ident", [C, C], F32, kind="ExternalInput")
    out_d = nc.dram_tensor("out", [C, P], F16, kind="ExternalOutput")

    Id = mybir.ActivationFunctionType.Identity

    with SplitDrainTileContext(nc) as tc:
      for _rep in range(reps):
        with ExitStack() as octx:
            cpool = octx.enter_context(tc.tile_pool(name="consts", bufs=1))

            # ---- mask first (gates phase A), then weights (needed in B) ----
            m_pool = octx.enter_context(tc.tile_pool(name="mA", bufs=1))
            MHALF = (NAP2 // 2) * 2 * KP
            m_all0 = m_pool.tile([CHA, MHALF], F8, tag="m0")
            nc.scalar.dma_start(m_all0[:], mask_d[:, :MHALF])
            m_all1 = m_pool.tile([CHA, MHALF], F8, tag="m1")
            nc.scalar.dma_start(m_all1[:], mask_d[:, MHALF:])
            abctx = octx.enter_context(ExitStack())
            xg_pool = abctx.enter_context(tc.tile_pool(name="xg", bufs=3))
            C2W = 2 * C   # 256 cols per DoubleRow chunk
            K2W = 2 * KP
            xg_tiles = {}
            wB_s = cpool.tile([C, WB_COLS], F32, tag="wB")
            nc.scalar.dma_start(wB_s[:], wB_d[:])
            wS_s = cpool.tile([Dh, H * C + 2 * H], F32, tag="wS")
            nc.scalar.dma_start(wS_s[:], wS_d[:])
            vb_s = cpool.tile([K, C], F32, tag="vb")
            nc.scalar.dma_start(vb_s[:], vb_d[:])
            ob_s = cpool.tile([C, 1], F32, tag="ob")
            nc.scalar.dma_start(ob_s[:], ob_d[:])
            ident_s = cpool.tile([C, C], F32, tag="ident")
            nc.scalar.dma_start(ident_s[:], ident_d[:])
            identh_s = cpool.tile([C, C], F16, tag="identh")
            nc.scalar.dma_start(identh_s[:], identh_d[:])
            # warm the Act tanh table while phase A runs
            tanh_warm = cpool.tile([C, 1], F32, tag="tanhw")
            nc.scalar.activation(tanh_warm[:], ob_s[:],
                                 mybir.ActivationFunctionType.Tanh)

            # ---------------- phase A: cavity pooling ----------------
            # featT[c,k] accumulated over 784 point-chunks in one PSUM bank.
            apool = abctx.enter_context(tc.tile_pool(name="a_ps", bufs=1,
                                                     space="PSUM"))
            featT_ps = apool.tile([C, KP], F32, tag="featT")
            with ExitStack() as actx:
                for g in range(NGA):
                    if g in xg_tiles:
                        xg_t = xg_tiles.pop(g)
                    else:
                        xg_t = xg_pool.tile([CHA, GA * C2W], F8, tag="xg")
                        eng = nc.sync if g % 2 == 0 else nc.gpsimd
                        eng.dma_start(
                            xg_t[:], xT_d[:, g * GA * C2W:(g + 1) * GA * C2W])
                    for j in range(GA):
                        ci = g * GA + j
                        mh, mi = divmod(ci, NAP2 // 2)
                        m_t = m_all0 if mh == 0 else m_all1
                        nc.tensor.matmul(
                            featT_ps[:],
                            lhsT=xg_t[:, j * C2W:(j + 1) * C2W]
                            .rearrange("p (i c) -> p i c", i=2),
                            rhs=m_t[:, mi * K2W:(mi + 1) * K2W]
                            .rearrange("p (i k) -> p i k", i=2),
                            start=(ci == 0),
                            stop=(ci == NAP2 - 1),
                            perf_mode=mybir.MatmulPerfMode.DoubleRow,
                        )

            # ---------------- phase B: MLP + attention (f32) ----------------
            with ExitStack() as bctx:
                sp = bctx.enter_context(
                    tc.tile_pool(name="sp_ps", bufs=2, space="PSUM"))

                featT = cpool.tile([C, K], F32, tag="featT_s")
                nc.vector.tensor_copy(featT[:], featT_ps[:, :K])

                # MLP: ph_all[:, 2k+h] = W1[k,h-half]^T @ featT[:,k]
                ph_all = sp.tile([C, 2 * K], F32, tag="sps")
                for k in range(K):
                    for hf in range(2):
                        nc.tensor.matmul(
                            ph_all[:, 2 * k + hf:2 * k + hf + 1],
                            lhsT=wB_s[:, W1_O + k * C2 + hf * C:
                                      W1_O + k * C2 + (hf + 1) * C],
                            rhs=featT[:, k:k + 1],
                            start=(k == 0 and hf == 0),
                            stop=(k == K - 1 and hf == 1),
                        )
                h_all = cpool.tile([C, 2 * K], F32, tag="h_all")
                nc.vector.tensor_add(h_all[:], ph_all[:],
                                     wB_s[:, B1_O:B1_O + 2 * K])
                nc.vector.tensor_scalar_max(h_all[:], h_all[:], 0.0)

                pp_all = sp.tile([C, K], F32, tag="sps")
                for k in range(K):
                    for hf in range(2):
                        nc.tensor.matmul(
                            pp_all[:, k:k + 1],
                            lhsT=wB_s[:, W2_O + (2 * k + hf) * C:
                                      W2_O + (2 * k + hf + 1) * C],
                            rhs=h_all[:, 2 * k + hf:2 * k + hf + 1],
                            start=(k == 0 and hf == 0),
                            stop=(k == K - 1 and hf == 1),
                        )
                pb_all = cpool.tile([C, K], F32, tag="pb_all")
                nc.vector.tensor_add(pb_all[:], pp_all[:],
                                     wB_s[:, B2_O:B2_O + K])
                procT = cpool.tile([C, K], F32, tag="procT")
                nc.scalar.activation(procT[:], pb_all[:],
                                     mybir.ActivationFunctionType.Tanh)

                # ---- attention over K=14 cavities ----
                # q/k in head-blocked layout [Dh, H*K]
                pq = sp.tile([Dh, H * K], F32, tag="sps")
                for h in range(H):
                    nc.tensor.matmul(pq[:, h * K:(h + 1) * K],
                                     lhsT=wB_s[:, WQ_O + h * Dh:
                                               WQ_O + (h + 1) * Dh],
                                     rhs=procT[:],
                                     start=(h == 0), stop=(h == H - 1))
                qh_s = cpool.tile([Dh, H * K], F32, tag="qT")
                for h in range(H):
                    nc.vector.tensor_scalar_add(
                        qh_s[:, h * K:(h + 1) * K],
                        pq[:, h * K:(h + 1) * K],
                        wS_s[:, H * C + h:H * C + h + 1])

                pk = sp.tile([Dh, H * K], F32, tag="sps")
                for h in range(H):
                    nc.tensor.matmul(pk[:, h * K:(h + 1) * K],
                                     lhsT=wB_s[:, WK_O + h * Dh:
                                               WK_O + (h + 1) * Dh],
                                     rhs=procT[:],
                                     start=(h == 0), stop=(h == H - 1))
                kh_s = cpool.tile([Dh, H * K], F32, tag="kT")
                for h in range(H):
                    nc.vector.tensor_scalar_add(
                        kh_s[:, h * K:(h + 1) * K],
                        pk[:, h * K:(h + 1) * K],
                        wS_s[:, H * C + H + h:H * C + H + h + 1])

                pv = sp.tile([K, C], F32, tag="sps")
                nc.tensor.matmul(pv[:], lhsT=procT[:],
                                 rhs=wB_s[:, WV_O:WV_O + C])
                v_s = cpool.tile([K, C], F32, tag="v")
                nc.vector.tensor_add(v_s[:], pv[:], vb_s[:])

                psc = sp.tile([K, H * K], F32, tag="sps")
                for h in range(H):
                    nc.tensor.matmul(
                        psc[:, h * K:(h + 1) * K],
                        lhsT=qh_s[:, h * K:(h + 1) * K],
                        rhs=kh_s[:, h * K:(h + 1) * K],
                        start=(h == 0),
                        stop=(h == H - 1),
                    )
                negmax = cpool.tile([K, H], F32, tag="negmax")
                nc.vector.tensor_reduce(
                    out=negmax[:],
                    in_=psc[:].rearrange("p (h j) -> p h j", j=K),
                    op=mybir.AluOpType.max,
                    axis=mybir.AxisListType.X,
                    negate=True,
                )
                esc = cpool.tile([K, H * K], F32, tag="esc")
                for h in range(H):
                    nc.scalar.activation(
                        esc[:, h * K:(h + 1) * K], psc[:, h * K:(h + 1) * K],
                        mybir.ActivationFunctionType.Exp,
                        bias=negmax[:, h:h + 1],
                    )
                ssum = cpool.tile([K, H], F32, tag="ssum")
                nc.vector.tensor_reduce(
                    out=ssum[:],
                    in_=esc[:].rearrange("p (h j) -> p h j", j=K),
                    op=mybir.AluOpType.add,
                    axis=mybir.AxisListType.X,
                )
                rinv = cpool.tile([K, H], F32, tag="rinv")
                nc.vector.reciprocal(rinv[:], ssum[:])
                for h in range(H):
                    nc.vector.tensor_scalar_mul(
                        esc[:, h * K:(h + 1) * K], esc[:, h * K:(h + 1) * K],
                        rinv[:, h:h + 1],
                    )

                pat = sp.tile([K, H * K], F32, tag="sps")
                for h in range(H):
                    nc.tensor.matmul(
                        pat[:, h * K:(h + 1) * K],
                        lhsT=esc[:, h * K:(h + 1) * K],
                        rhs=ident_s[:K, :K],
                        is_transpose=True,
                        start=(h == 0),
                        stop=(h == H - 1),
                    )
                at_s = cpool.tile([K, H * K], F32, tag="at")
                nc.vector.tensor_copy(at_s[:], pat[:])

                # o in head-blocked layout [Dh, H*K]
                po = sp.tile([Dh, H * K], F32, tag="sps")
                for h in range(H):
                    nc.tensor.matmul(
                        po[:, h * K:(h + 1) * K],
                        lhsT=v_s[:, h * Dh:(h + 1) * Dh],
                        rhs=at_s[:, h * K:(h + 1) * K],
                        start=(h == 0),
                        stop=(h == H - 1),
                    )
                o_s = cpool.tile([Dh, H * K], F32, tag="o")
                nc.vector.tensor_copy(o_s[:], po[:])

                # attT[e,i] = sum_h Wo[:, h-block] @ o_head_h  (accumulate)
                patt = sp.tile([C, K], F32, tag="sps")
                for h in range(H):
                    nc.tensor.matmul(patt[:],
                                     lhsT=wS_s[:, h * C:(h + 1) * C],
                                     rhs=o_s[:, h * K:(h + 1) * K],
                                     start=(h == 0), stop=(h == H - 1))
                attT_s = cpool.tile([C, K], F32, tag="attT")
                nc.vector.tensor_scalar_add(attT_s[:], patt[:], ob_s[:])

                pak = sp.tile([K, C], F32, tag="sps")
                nc.tensor.transpose(pak[:], attT_s[:], ident_s[:])
                ak_s = cpool.tile([K, C], F16, tag="ak")
                nc.vector.tensor_copy(ak_s[:], pak[:])
            abctx.close()   # free phase A/B PSUM banks for phase C

            # ---------------- phase C: gather-add ----------------
            # 4096-pt xc/out DMA super-blocks (Pool / SP queues), 8192-pt
            # fp8 onehot batches (Act queue).  Per 512-pt chunk: gather
            # matmul; chunks 0,1 add x on DVE straight from PSUM, chunks
            # 2,3 accumulate x on PE (identity matmul) and copy out on Act.
            OB = 2 * CB              # 4096
            NSB = (NB + 1) // 2      # 25, last super-block has 1 block
            OHB = 4 * CB             # 8192
            with ExitStack() as cctx:
                xc_pool = cctx.enter_context(tc.tile_pool(name="xc", bufs=7))
                oh_pool = cctx.enter_context(tc.tile_pool(name="oh", bufs=4))
                oc_pool = cctx.enter_context(tc.tile_pool(name="oc", bufs=4))
                pc_pool = cctx.enter_context(
                    tc.tile_pool(name="pc", bufs=4, space="PSUM"))

                NOHB = (NB + 3) // 4          # 13 onehot batches
                LA = 5                        # xc load lookahead

                def issue_xc(i):
                    s0 = i * OB
                    sw = min(OB, PT - s0)
                    t = xc_pool.tile([C, OB], F16, tag="xc")
                    nc.gpsimd.dma_start(t[:, :sw], xc_d[:, s0:s0 + sw])
                    return t

                def issue_oh(g):
                    o0 = g * OHB
                    ow = min(OHB, PT - o0)
                    t = oh_pool.tile([K, OHB], F8, tag="oh")
                    nc.scalar.dma_start(t[:, :ow], onehot_d[:, o0:o0 + ow])
                    return t

                xc_tiles = {i: issue_xc(i) for i in range(min(LA, NSB))}
                oh_tiles = {g: issue_oh(g) for g in range(min(3, NOHB))}

                for sb in range(NSB):
                    if sb + LA < NSB:
                        xc_tiles[sb + LA] = issue_xc(sb + LA)
                    s0 = sb * OB
                    xc_t = xc_tiles.pop(sb)
                    oc_t = oc_pool.tile([C, OB], F16, tag="ocd")
                    for jb in (0, 1):
                        b0 = s0 + jb * CB
                        if b0 >= P:
                            break
                        gi = sb * 2 + jb
                        g = gi // 4
                        if g + 3 < NOHB and g + 3 not in oh_tiles and gi % 4 == 0:
                            oh_tiles[g + 3] = issue_oh(g + 3)
                        oh_t = oh_tiles[g]
                        ohoff = (gi % 4) * CB
                        xoff = jb * CB
                        # two-bank [C, 1024] psum tiles: one DVE add (chunks
                        # 0-1) and one Act copy (chunks 2-3) per 2048-pt block
                        for j2 in (0, 1):
                            j0 = j2 * 2 * CHC
                            pc_t = pc_pool.tile([C, 2 * CHC], F32, tag="pc")
                            for jj in (0, 1):
                                j = j0 + jj * CHC
                                nc.tensor.matmul(
                                    pc_t[:, jj * CHC:(jj + 1) * CHC],
                                    lhsT=ak_s[:],
                                    rhs=oh_t[:, ohoff + j:ohoff + j + CHC],
                                    start=True, stop=(j2 == 0))
                                if j2 == 1:
                                    nc.tensor.matmul(
                                        pc_t[:, jj * CHC:(jj + 1) * CHC],
                                        lhsT=identh_s[:],
                                        rhs=xc_t[:, xoff + j:xoff + j + CHC],
                                        start=False, stop=True)
                            if j2 == 0:
                                nc.vector.tensor_add(
                                    oc_t[:, xoff + j0:xoff + j0 + 2 * CHC],
                                    pc_t[:],
                                    xc_t[:, xoff + j0:xoff + j0 + 2 * CHC])
                            else:
                                nc.scalar.activation(
                                    oc_t[:, xoff + j0:xoff + j0 + 2 * CHC],
                                    pc_t[:], Id)
                        bw = min(CB, P - b0)
                        nc.sync.dma_start(out_d[:, b0:b0 + bw],
                                          oc_t[:, xoff:xoff + bw])

    return nc


def prep_host(points, cavities, w1, b1, w2, b2, in_w, in_b, out_w, out_b):
    """Geometry + weight preprocessing (pure numpy, no x dependence)."""
    points = np.asarray(points, np.float32)
    cavities = np.asarray(cavities, np.float32)
    d = np.sqrt(
        ((points[None, :, :] - cavities[:, None, :]) ** 2).sum(-1, dtype=np.float32)
    ).astype(np.float32)                                   # [K, P]
    mask = (d < RADIUS).astype(np.float32)                 # [K, P]
    counts = mask.sum(axis=1, dtype=np.float32)            # [K]
    inv = np.where(counts > 0, 1.0 / np.maximum(counts, 1.0), 0.0).astype(np.float32)

    # DoubleRow mask: maskA[p, ci*2KP + i*KP + k] = mask[k, ci*256 + 2p + i]
    maskA = np.zeros((PT, KP), np.float32)
    maskA[:P, :K] = mask.T
    maskA = (maskA.reshape(NAP2, CHA, 2, KP).transpose(1, 0, 2, 3)
             .reshape(CHA, NAP2 * 2 * KP)).astype(NP_F8)

    nearest = np.argmin(d, axis=0)                         # [P]
    onehot = np.zeros((K, PT), NP_F8)
    onehot[nearest, np.arange(P)] = 1.0

    w1 = np.asarray(w1, np.float32)
    w2 = np.asarray(w2, np.float32)
    scale = np.float32(1.0 / np.sqrt(Dh))

    wB = np.zeros((C, WB_COLS), np.float32)
    # w1t scaled by inv[k]: wB[c, W1_O + k*C2 + d] = w1[k, d, c] * inv[k]
    w1t = (w1.transpose(0, 2, 1) * inv[:, None, None]).astype(np.float32)  # [K,C,C2]
    wB[:, W1_O:W1_O + K * C2] = w1t.transpose(1, 0, 2).reshape(C, K * C2)
    # w2t halves: wB[d, W2_O + (2k+h)*C + c] = w2[k, c, h*C+d]
    w2t = w2.transpose(0, 2, 1).reshape(K, 2, C, C)        # [K,2,C(d),C(c)]
    wB[:, W2_O:W2_O + K * 2 * C] = w2t.transpose(2, 0, 1, 3).reshape(C, K * 2 * C)
    b1 = np.asarray(b1, np.float32)
    wB[:, B1_O:B1_O + 2 * K] = b1.reshape(K, 2, C).transpose(2, 0, 1).reshape(C, 2 * K)
    wB[:, B2_O:B2_O + K] = np.asarray(b2, np.float32).T
    in_w = np.asarray(in_w, np.float32)
    wB[:, WQ_O:WQ_O + C] = in_w[0:C].T * scale
    wB[:, WK_O:WK_O + C] = in_w[C:2 * C].T
    wB[:, WV_O:WV_O + C] = in_w[2 * C:3 * C].T

    in_b = np.asarray(in_b, np.float32)
    # wS: wo_heads[d, h*C+e] = out_w[e, h*Dh+d], then qb/kb head-blocked
    wS = np.concatenate([
        np.asarray(out_w, np.float32).reshape(C, H, Dh)
        .transpose(2, 1, 0).reshape(Dh, H * C),
        (in_b[0:C] * scale).reshape(H, Dh).T,              # [Dh, H]
        in_b[C:2 * C].reshape(H, Dh).T,
    ], axis=1)                                             # [Dh, H*C+2H]

    fp = {
        "maskA": np.ascontiguousarray(maskA),
        "onehot": np.ascontiguousarray(onehot),
        "wB": np.ascontiguousarray(wB),
        "wS": np.ascontiguousarray(wS),
        "vb": np.ascontiguousarray(np.tile(in_b[2 * C:3 * C], (K, 1))),
        "ob": np.ascontiguousarray(np.asarray(out_b, np.float32).reshape(C, 1)),
        "ident": np.eye(C, dtype=np.float32),
        "identh": np.eye(C, dtype=np.float16),
    }
    return fp


def prep_x(xb):
    """Per-batch x prep: fp8 DoubleRow-transposed copy (phase A) + f16 copy
    (phase C).  xT[p, ci*2C + i*C + c] = x[c, ci*256 + 2p + i]."""
    xb = np.asarray(xb, np.float32)
    xpad = np.zeros((C, PT), np.float32)
    xpad[:, :P] = xb
    xT = np.ascontiguousarray(
        xpad.reshape(C, NAP2, CHA, 2).transpose(2, 1, 3, 0)
    ).reshape(CHA, NAP2 * 2 * C)
    return {
        "xT": xT.astype(NP_F8),
        "xc": xpad.astype(np.float16),
    }


def make_in_maps(x, fp):
    x = np.asarray(x, np.float32)
    return [dict(fp, **prep_x(x[b])) for b in range(B)]


_PROGRAM = None


def kernel(x, points, cavities, w1, b1, w2, b2, in_w, in_b, out_w, out_b):
    global _PROGRAM
    fp = prep_host(points, cavities, w1, b1, w2, b2, in_w, in_b, out_w, out_b)
    if _PROGRAM is None:
        _PROGRAM = build_program()
    nc = _PROGRAM
    in_maps = make_in_maps(x, fp)
    res = run_bass_kernel_spmd(nc, in_maps, list(range(B)))
    out = np.stack([np.asarray(res.results[b]["out"], np.float32)
                    for b in range(B)], axis=0)
    return out
